# revision 11
# baseline (speedup 1.0000x reference)
"""Dcls2_1d (dilated conv with learnable row spacings) on 8 trn2 NeuronCores.

Strategy: data-parallel over batch (16 -> 2 images/core). Host constructs the
dense (O, I, 7, 3) scattered kernel (exact port of the reference bilinear
scatter, ~0.7 MFLOP) and pads x; each core runs the conv as an implicit GEMM:
for every output chunk of 512 pixels, 21 taps are accumulated in PSUM via
matmuls contracting over C_in=128 (the partition dim), then the bias add is
fused into the PSUM->SBUF evacuation on the scalar engine.

float32r matmuls stream 1 row/cycle (vs 4 for float32) at ~tf32 precision.
Input DMAs are ordered so the first accumulation group's operands land first
(spread over all 16 DMA queues), and a short burst of dummy matmuls warms the
PE clock (HAM) while the real inputs are still in flight.
"""
import os
import sys
import time

sys.path.insert(0, "/opt/trn_rl_repo")

import ml_dtypes
import numpy as np

import concourse.bass as bass
import concourse.tile as tile
from concourse import bacc, mybir
from concourse import bass_utils

# ---- problem constants (hardcoded per contract) ----
K_H, K_W = 3, 3
LIM = 2            # DIL // 2
KH_EFF = 7         # K_H + 2 * LIM
PAD_H, PAD_W = 3, 1
B, CIN, H, W = 16, 128, 64, 64
COUT = 256
N_CORES = 8
BPC = B // N_CORES                  # images per core
HP, WP = H + 2 * PAD_H, W + 2 * PAD_W   # 70, 66
NPIX = H * W                        # 4096
CHUNK = 512                         # output pixels per PSUM bank
NCHUNK = NPIX // CHUNK              # 8
RPC = CHUNK // W                    # rows per chunk: 8
NTAPS = KH_EFF * K_W                # 21
OH = COUT // 128                    # 2 halves of out channels

DT = os.environ.get("DCLS_DT", "f32r")          # f32r | fp16 | bf16 | f32
ORDER = os.environ.get("DCLS_ORDER", "chunk")    # chunk | tap
WARMUP = int(os.environ.get("DCLS_WARMUP", "10"))
_MM_DT = {"f32r": mybir.dt.float32r, "fp16": mybir.dt.float16,
          "bf16": mybir.dt.bfloat16, "f32": mybir.dt.float32}[DT]
_NP_DT = {"f32r": np.float32, "fp16": np.float16,
          "bf16": ml_dtypes.bfloat16, "f32": np.float32}[DT]

if os.environ.get("DCLS_LDWOPT", "0") == "1":
    _orig_run_command = bass_utils.run_command

    def _patched_run_command(cmd, **kw):
        cmd = ["--enable-ldw-opt=true" if c == "--enable-ldw-opt=false" else c
               for c in cmd]
        return _orig_run_command(cmd, **kw)

    bass_utils.run_command = _patched_run_command

_NC_CACHE = None
_last_in_maps = None  # stashed for test.py's profiled re-run


def _build_kernel_np(weight: np.ndarray, P1: np.ndarray) -> np.ndarray:
    """Exact numpy port of reference.build_kernel (fp32)."""
    weight = weight.astype(np.float32, copy=False)
    kh = np.arange(K_H, dtype=np.float32)[None, None, :, None]
    pos = kh + LIM + np.clip(P1.astype(np.float32, copy=False), -LIM, LIM)
    p0 = np.floor(pos)
    frac = pos - p0
    p0i = p0.astype(np.int32)
    rng = np.arange(KH_EFF, dtype=np.int32)
    oh0 = (p0i[..., None] == rng).astype(np.float32)
    oh1 = ((p0i + 1)[..., None] == rng).astype(np.float32)
    return (
        np.einsum("oihw,oihwk->oikw", weight * (1.0 - frac), oh0)
        + np.einsum("oihw,oihwk->oikw", weight * frac, oh1)
    ).astype(np.float32)


def _splits(total, n):
    """n near-equal [lo, hi) column ranges covering [0, total)."""
    step = (total + n - 1) // n
    return [(j, min(j + step, total)) for j in range(0, total, step)]


def _build_bass():
    mmdt = _MM_DT
    f32 = mybir.dt.float32
    nc = bacc.Bacc("TRN2", target_bir_lowering=False, debug=False,
                   num_devices=N_CORES)
    x_d = nc.dram_tensor("x", [BPC, CIN, HP * WP], mmdt,
                         kind="ExternalInput").ap()
    # oh-major weight layout: [i, (oh, kh, kw, o128)]
    k_d = nc.dram_tensor("k", [CIN, OH * NTAPS * 128], mmdt,
                         kind="ExternalInput").ap()
    b_d = nc.dram_tensor("b", [OH, 128, 1], f32, kind="ExternalInput").ap()
    o_d = nc.dram_tensor("o", [BPC, OH, 128, NPIX], f32,
                         kind="ExternalOutput").ap()

    HEAD_ROWS = RPC + KH_EFF - 1            # x rows needed by first chunk: 14
    HEAD = HEAD_ROWS * WP                   # 924 cols

    # DMA descriptor issue costs ~0.6us on an engine queue; spread issues
    # over four otherwise-idle engine queues so they go out in parallel.
    _rr = [0]

    def dma(engines, dst, src):
        eng = engines[_rr[0] % len(engines)]
        _rr[0] += 1
        eng.dma_start(dst, src)

    with tile.TileContext(nc) as tc:
        with tc.tile_pool(name="xp", bufs=1) as xp, \
             tc.tile_pool(name="kp", bufs=1) as kp, \
             tc.tile_pool(name="bp", bufs=1) as bp, \
             tc.tile_pool(name="wu", bufs=1) as wu, \
             tc.tile_pool(name="ps", bufs=8, space="PSUM") as ps, \
             tc.tile_pool(name="op", bufs=4) as op:

            kt = kp.tile([CIN, OH * NTAPS * 128], mmdt, tag="k")
            bt = bp.tile([128, OH], f32, tag="bias")
            xts = [xp.tile([CIN, HP * WP], mmdt, tag=f"x{n}", name=f"x{n}")
                   for n in range(BPC)]

            # warmup tile for the PE clock (HAM) ramp: memset-fed fp32
            # (no DMA deps) so the dummy matmuls run while the real inputs
            # are still in flight; their PSUM output is never read
            wt = None
            if WARMUP:
                wt = wu.tile([128, 128], f32, tag="warm")
                nc.vector.memset(wt[:], 0.0)

            # --- input DMAs, priority-ordered, issued from 4 engines in
            # parallel, spread over the 16 HW queues ---
            ie = [nc.sync, nc.gpsimd, nc.scalar]
            # 1) first rows of image 0 (first matmul needs them + tap0 weights)
            for lo, hi in _splits(HEAD, 8):
                dma(ie, xts[0][:, lo:hi], x_d[0][:, lo:hi])
            # 2) weights for the first oh half, fine-grained so taps stream in
            for lo, hi in _splits(NTAPS * 128, 16):
                dma(ie, kt[:, lo:hi], k_d[:, lo:hi])
            # 3) rest of image 0
            for lo, hi in _splits(HP * WP - HEAD, 5):
                dma(ie, xts[0][:, HEAD + lo:HEAD + hi],
                    x_d[0][:, HEAD + lo:HEAD + hi])
            # 4) bias, second weight half, remaining images
            for h in range(OH):
                dma(ie, bt[:, h:h + 1], b_d[h])
            for lo, hi in _splits(NTAPS * 128, 8):
                off = NTAPS * 128
                dma(ie, kt[:, off + lo:off + hi], k_d[:, off + lo:off + hi])
            for n in range(1, BPC):
                for lo, hi in _splits(HP * WP, 6):
                    dma(ie, xts[n][:, lo:hi], x_d[n][:, lo:hi])

            # --- HAM warmup: dummy matmuls while inputs stream in ---
            for _ in range(WARMUP):
                pw = ps.tile([128, 128], f32, tag="acc")
                nc.tensor.matmul(pw[:], wt[:], wt[:], start=True,
                                 stop=True)

            # --- the conv ---
            def do_group(n, h, c, xv):
                pt = ps.tile([128, CHUNK], f32, tag="acc")
                y0 = c * RPC
                for t, (kh, kw) in enumerate(
                        (kh, kw) for kh in range(KH_EFF)
                        for kw in range(K_W)):
                    rhs = xv[:, y0 + kh:y0 + kh + RPC, kw:kw + W]
                    off = ((h * KH_EFF + kh) * K_W + kw) * 128
                    nc.tensor.matmul(pt[:], kt[:, off:off + 128], rhs,
                                     start=(t == 0), stop=(t == NTAPS - 1))
                ot = op.tile([128, CHUNK], f32, tag="out")
                nc.scalar.activation(ot[:], pt[:],
                                     mybir.ActivationFunctionType.Identity,
                                     bias=bt[:, h:h + 1])
                # split the store so the flush of the last chunk isn't
                # bottlenecked on a single ~22GB/s DMA queue; the very last
                # store goes 8-way on the HW queues (SW queues drain slowly)
                last = (n == BPC - 1 and h == OH - 1 and c == NCHUNK - 1)
                oe = [nc.sync, nc.scalar] if last else [nc.sync, nc.gpsimd]
                for lo, hi in _splits(CHUNK, 8 if last else 2):
                    dma(oe, o_d[n, h][:, c * CHUNK + lo:c * CHUNK + hi],
                        ot[:, lo:hi])

            def do_block_tap_outer(n, h, xv):
                pts = [ps.tile([128, CHUNK], f32, tag="acc",
                               name=f"acc_{n}_{h}_{c}")
                       for c in range(NCHUNK)]
                for t, (kh, kw) in enumerate(
                        (kh, kw) for kh in range(KH_EFF)
                        for kw in range(K_W)):
                    off = ((h * KH_EFF + kh) * K_W + kw) * 128
                    for c in range(NCHUNK):
                        rhs = xv[:, c * RPC + kh:c * RPC + kh + RPC, kw:kw + W]
                        nc.tensor.matmul(pts[c][:], kt[:, off:off + 128], rhs,
                                         start=(t == 0),
                                         stop=(t == NTAPS - 1))
                for c in range(NCHUNK):
                    ot = op.tile([128, CHUNK], f32, tag="out")
                    nc.scalar.activation(ot[:], pts[c][:],
                                         mybir.ActivationFunctionType.Identity,
                                         bias=bt[:, h:h + 1])
                    last = (n == BPC - 1 and h == OH - 1 and c == NCHUNK - 1)
                    oe = [nc.sync, nc.gpsimd]
                    for lo, hi in _splits(CHUNK, 4 if last else 2):
                        dma(oe, o_d[n, h][:, c * CHUNK + lo:c * CHUNK + hi],
                            ot[:, lo:hi])

            for n in range(BPC):
                xv = xts[n][:].rearrange("p (h w) -> p h w", h=HP)
                for h in range(OH):
                    if ORDER == "tap":
                        do_block_tap_outer(n, h, xv)
                    else:
                        for c in range(NCHUNK):
                            do_group(n, h, c, xv)
    t0 = time.time()
    nc.compile()
    print(f"[kernel] bacc compile: {time.time()-t0:.1f}s", file=sys.stderr)
    return nc


def kernel(x: np.ndarray, weight: np.ndarray, bias: np.ndarray,
           P: np.ndarray) -> np.ndarray:
    global _NC_CACHE, _last_in_maps
    x = np.asarray(x, dtype=np.float32)
    weight = np.asarray(weight, dtype=np.float32)
    bias = np.asarray(bias, dtype=np.float32)
    P = np.asarray(P, dtype=np.float32)

    K = _build_kernel_np(weight, P[0])                    # (O, I, 7, 3)
    # device layout: [i, (oh, kh, kw, o128)]
    k_dev = np.ascontiguousarray(
        K.reshape(OH, 128, CIN, KH_EFF, K_W)
        .transpose(2, 0, 3, 4, 1)
        .reshape(CIN, OH * NTAPS * 128)).astype(_NP_DT)

    xpad = np.zeros((B, CIN, HP, WP), np.float32)
    xpad[:, :, PAD_H:PAD_H + H, PAD_W:PAD_W + W] = x
    xpad = xpad.reshape(B, CIN, HP * WP).astype(_NP_DT)

    b_dev = np.ascontiguousarray(bias.reshape(OH, 128, 1))

    if _NC_CACHE is None:
        t0 = time.time()
        _NC_CACHE = _build_bass()
        print(f"[kernel] build+compile total: {time.time()-t0:.1f}s",
              file=sys.stderr)

    in_maps = [
        {"x": np.ascontiguousarray(xpad[i * BPC:(i + 1) * BPC]),
         "k": k_dev, "b": b_dev}
        for i in range(N_CORES)
    ]
    _last_in_maps = in_maps
    t0 = time.time()
    res = bass_utils.run_bass_kernel_spmd(
        _NC_CACHE, in_maps, core_ids=list(range(N_CORES)))
    print(f"[kernel] run (incl. walrus compile on first call): "
          f"{time.time()-t0:.1f}s", file=sys.stderr)
    out = np.concatenate(
        [res.results[i]["o"].reshape(BPC, COUT, H, W)
         for i in range(N_CORES)], axis=0)
    return out


# revision 16
# speedup vs baseline: 1.5340x; 1.5340x over previous
"""Dcls2_1d (dilated conv with learnable row spacings) on 8 trn2 NeuronCores.

Strategy: data-parallel over batch (16 -> 2 images/core). Host constructs the
dense (O, I, 7, 3) scattered kernel (exact port of the reference bilinear
scatter, ~0.7 MFLOP) and pads x; each core runs the conv as an implicit GEMM:
for every output chunk of 512 pixels, 21 taps are accumulated in PSUM via
matmuls contracting over C_in=128 (the partition dim), then the bias add is
fused into the PSUM->SBUF evacuation on the scalar engine.

float32r matmuls stream 1 row/cycle (vs 4 for float32) at ~tf32 precision.
Input DMAs are ordered so the first accumulation group's operands land first
(spread over all 16 DMA queues), and a short burst of dummy matmuls warms the
PE clock (HAM) while the real inputs are still in flight.
"""
import os
import sys
import time

sys.path.insert(0, "/opt/trn_rl_repo")

import ml_dtypes
import numpy as np

import concourse.bass as bass
import concourse.tile as tile
from concourse import bacc, mybir
from concourse import bass_utils

# ---- problem constants (hardcoded per contract) ----
K_H, K_W = 3, 3
LIM = 2            # DIL // 2
KH_EFF = 7         # K_H + 2 * LIM
PAD_H, PAD_W = 3, 1
B, CIN, H, W = 16, 128, 64, 64
COUT = 256
N_CORES = 8
BPC = B // N_CORES                  # images per core
HP, WP = H + 2 * PAD_H, W + 2 * PAD_W   # 70, 66
NPIX = H * W                        # 4096
CHUNK = 512                         # output pixels per PSUM bank
NCHUNK = NPIX // CHUNK              # 8
RPC = CHUNK // W                    # rows per chunk: 8
NTAPS = KH_EFF * K_W                # 21
OH = COUT // 128                    # 2 halves of out channels

DT = os.environ.get("DCLS_DT", "fp16")          # f32r | fp16 | bf16 | f32
ALGO = os.environ.get("DCLS_ALGO", "wino")       # wino | direct
ORDER = os.environ.get("DCLS_ORDER", "chunk")    # chunk | tap
WARMUP = int(os.environ.get("DCLS_WARMUP", "10"))
_MM_DT = {"f32r": mybir.dt.float32r, "fp16": mybir.dt.float16,
          "bf16": mybir.dt.bfloat16, "f32": mybir.dt.float32}[DT]
_NP_DT = {"f32r": np.float32, "fp16": np.float16,
          "bf16": ml_dtypes.bfloat16, "f32": np.float32}[DT]

if os.environ.get("DCLS_LDWOPT", "0") == "1":
    _orig_run_command = bass_utils.run_command

    def _patched_run_command(cmd, **kw):
        cmd = ["--enable-ldw-opt=true" if c == "--enable-ldw-opt=false" else c
               for c in cmd]
        return _orig_run_command(cmd, **kw)

    bass_utils.run_command = _patched_run_command

_NC_CACHE = None
_last_in_maps = None  # stashed for test.py's profiled re-run


def _build_kernel_np(weight: np.ndarray, P1: np.ndarray) -> np.ndarray:
    """Exact numpy port of reference.build_kernel (fp32)."""
    weight = weight.astype(np.float32, copy=False)
    kh = np.arange(K_H, dtype=np.float32)[None, None, :, None]
    pos = kh + LIM + np.clip(P1.astype(np.float32, copy=False), -LIM, LIM)
    p0 = np.floor(pos)
    frac = pos - p0
    p0i = p0.astype(np.int32)
    rng = np.arange(KH_EFF, dtype=np.int32)
    oh0 = (p0i[..., None] == rng).astype(np.float32)
    oh1 = ((p0i + 1)[..., None] == rng).astype(np.float32)
    return (
        np.einsum("oihw,oihwk->oikw", weight * (1.0 - frac), oh0)
        + np.einsum("oihw,oihwk->oikw", weight * frac, oh1)
    ).astype(np.float32)


def _splits(total, n):
    """n near-equal [lo, hi) column ranges covering [0, total)."""
    step = (total + n - 1) // n
    return [(j, min(j + step, total)) for j in range(0, total, step)]


def _build_bass():
    mmdt = _MM_DT
    f32 = mybir.dt.float32
    nc = bacc.Bacc("TRN2", target_bir_lowering=False, debug=False,
                   num_devices=N_CORES)
    x_d = nc.dram_tensor("x", [BPC, CIN, HP * WP], mmdt,
                         kind="ExternalInput").ap()
    # oh-major weight layout: [i, (oh, kh, kw, o128)]
    k_d = nc.dram_tensor("k", [CIN, OH * NTAPS * 128], mmdt,
                         kind="ExternalInput").ap()
    b_d = nc.dram_tensor("b", [OH, 128, 1], f32, kind="ExternalInput").ap()
    o_d = nc.dram_tensor("o", [BPC, OH, 128, NPIX], f32,
                         kind="ExternalOutput").ap()

    HEAD_ROWS = RPC + KH_EFF - 1            # x rows needed by first chunk: 14
    HEAD = HEAD_ROWS * WP                   # 924 cols

    # DMA descriptor issue costs ~0.6us on an engine queue; spread issues
    # over four otherwise-idle engine queues so they go out in parallel.
    _rr = [0]

    def dma(engines, dst, src):
        eng = engines[_rr[0] % len(engines)]
        _rr[0] += 1
        eng.dma_start(dst, src)

    with tile.TileContext(nc) as tc:
        with tc.tile_pool(name="xp", bufs=1) as xp, \
             tc.tile_pool(name="kp", bufs=1) as kp, \
             tc.tile_pool(name="bp", bufs=1) as bp, \
             tc.tile_pool(name="wu", bufs=1) as wu, \
             tc.tile_pool(name="ps", bufs=8, space="PSUM") as ps, \
             tc.tile_pool(name="op", bufs=4) as op:

            kt = kp.tile([CIN, OH * NTAPS * 128], mmdt, tag="k")
            bt = bp.tile([128, OH], f32, tag="bias")
            xts = [xp.tile([CIN, HP * WP], mmdt, tag=f"x{n}", name=f"x{n}")
                   for n in range(BPC)]

            # warmup tile for the PE clock (HAM) ramp: memset-fed fp32
            # (no DMA deps) so the dummy matmuls run while the real inputs
            # are still in flight; their PSUM output is never read
            wt = None
            if WARMUP:
                wt = wu.tile([128, 128], f32, tag="warm")
                nc.vector.memset(wt[:], 0.0)

            # --- input DMAs, priority-ordered, issued from 4 engines in
            # parallel, spread over the 16 HW queues ---
            ie = [nc.sync, nc.gpsimd, nc.scalar]
            # 1) first rows of image 0 (first matmul needs them + tap0 weights)
            for lo, hi in _splits(HEAD, 8):
                dma(ie, xts[0][:, lo:hi], x_d[0][:, lo:hi])
            # 2) weights for the first oh half, fine-grained so taps stream in
            for lo, hi in _splits(NTAPS * 128, 16):
                dma(ie, kt[:, lo:hi], k_d[:, lo:hi])
            # 3) rest of image 0
            for lo, hi in _splits(HP * WP - HEAD, 5):
                dma(ie, xts[0][:, HEAD + lo:HEAD + hi],
                    x_d[0][:, HEAD + lo:HEAD + hi])
            # 4) bias, second weight half, remaining images
            for h in range(OH):
                dma(ie, bt[:, h:h + 1], b_d[h])
            for lo, hi in _splits(NTAPS * 128, 8):
                off = NTAPS * 128
                dma(ie, kt[:, off + lo:off + hi], k_d[:, off + lo:off + hi])
            for n in range(1, BPC):
                for lo, hi in _splits(HP * WP, 6):
                    dma(ie, xts[n][:, lo:hi], x_d[n][:, lo:hi])

            # --- HAM warmup: dummy matmuls while inputs stream in ---
            for _ in range(WARMUP):
                pw = ps.tile([128, 128], f32, tag="acc")
                nc.tensor.matmul(pw[:], wt[:], wt[:], start=True,
                                 stop=True)

            # --- the conv ---
            def do_group(n, h, c, xv):
                pt = ps.tile([128, CHUNK], f32, tag="acc")
                y0 = c * RPC
                for t, (kh, kw) in enumerate(
                        (kh, kw) for kh in range(KH_EFF)
                        for kw in range(K_W)):
                    rhs = xv[:, y0 + kh:y0 + kh + RPC, kw:kw + W]
                    off = ((h * KH_EFF + kh) * K_W + kw) * 128
                    nc.tensor.matmul(pt[:], kt[:, off:off + 128], rhs,
                                     start=(t == 0), stop=(t == NTAPS - 1))
                ot = op.tile([128, CHUNK], f32, tag="out")
                nc.scalar.activation(ot[:], pt[:],
                                     mybir.ActivationFunctionType.Identity,
                                     bias=bt[:, h:h + 1])
                # split the store so the flush of the last chunk isn't
                # bottlenecked on a single ~22GB/s DMA queue; the very last
                # store goes 8-way on the HW queues (SW queues drain slowly)
                last = (n == BPC - 1 and h == OH - 1 and c == NCHUNK - 1)
                oe = [nc.sync, nc.scalar] if last else [nc.sync, nc.gpsimd]
                for lo, hi in _splits(CHUNK, 8 if last else 2):
                    dma(oe, o_d[n, h][:, c * CHUNK + lo:c * CHUNK + hi],
                        ot[:, lo:hi])

            def do_block_tap_outer(n, h, xv):
                pts = [ps.tile([128, CHUNK], f32, tag="acc",
                               name=f"acc_{n}_{h}_{c}")
                       for c in range(NCHUNK)]
                for t, (kh, kw) in enumerate(
                        (kh, kw) for kh in range(KH_EFF)
                        for kw in range(K_W)):
                    off = ((h * KH_EFF + kh) * K_W + kw) * 128
                    for c in range(NCHUNK):
                        rhs = xv[:, c * RPC + kh:c * RPC + kh + RPC, kw:kw + W]
                        nc.tensor.matmul(pts[c][:], kt[:, off:off + 128], rhs,
                                         start=(t == 0),
                                         stop=(t == NTAPS - 1))
                for c in range(NCHUNK):
                    ot = op.tile([128, CHUNK], f32, tag="out")
                    nc.scalar.activation(ot[:], pts[c][:],
                                         mybir.ActivationFunctionType.Identity,
                                         bias=bt[:, h:h + 1])
                    last = (n == BPC - 1 and h == OH - 1 and c == NCHUNK - 1)
                    oe = [nc.sync, nc.gpsimd]
                    for lo, hi in _splits(CHUNK, 4 if last else 2):
                        dma(oe, o_d[n, h][:, c * CHUNK + lo:c * CHUNK + hi],
                            ot[:, lo:hi])

            for n in range(BPC):
                xv = xts[n][:].rearrange("p (h w) -> p h w", h=HP)
                for h in range(OH):
                    if ORDER == "tap":
                        do_block_tap_outer(n, h, xv)
                    else:
                        for c in range(NCHUNK):
                            do_group(n, h, c, xv)
    t0 = time.time()
    nc.compile()
    print(f"[kernel] bacc compile: {time.time()-t0:.1f}s", file=sys.stderr)
    return nc


NJ = 4                       # Winograd F(2,3) points over kw
PAIRS = W // 2               # output column pairs: 32
STRIPS = [(0, 15), (15, 30), (30, 45), (45, 60), (60, 64)]
RB = [(0, 18), (18, 36), (36, 54), (54, 70)]   # input-transform row blocks


def _build_bass_wino():
    """Winograd F(2,3) over the width taps: out cols (2p, 2p+1) come from
    4 multiply-points j on input cols (2p..2p+3), so the PE streams 4/6 of
    the direct method's columns. Transforms run on the otherwise-idle
    DVE (input, output assembly) and ACT (bias) engines.

      W0 = d0-d2, W1 = d1+d2, W2 = d2-d1, W3 = d1-d3     (input, DVE)
      o_even = m0+m1+m2,  o_odd = m1-m2-m3               (output, DVE)
    """
    mmdt = _MM_DT
    f32 = mybir.dt.float32
    nc = bacc.Bacc("TRN2", target_bir_lowering=False, debug=False,
                   num_devices=N_CORES)
    x_d = nc.dram_tensor("x", [BPC, CIN, HP * WP], mmdt,
                         kind="ExternalInput").ap()
    # transformed weights: [i, (oh, j, kh, o128)]
    KCOLS = OH * NJ * KH_EFF * 128
    k_d = nc.dram_tensor("k", [CIN, KCOLS], mmdt, kind="ExternalInput").ap()
    b_d = nc.dram_tensor("b", [OH, 128, 1], f32, kind="ExternalInput").ap()
    o_d = nc.dram_tensor("o", [BPC, OH, 128, NPIX], f32,
                         kind="ExternalOutput").ap()

    _rr = [0]

    def dma(engines, dst, src):
        eng = engines[_rr[0] % len(engines)]
        _rr[0] += 1
        eng.dma_start(dst, src)

    HEAD = RB[0][1] * WP      # x cols needed by the first transform block

    with tile.TileContext(nc) as tc:
        with tc.tile_pool(name="xp", bufs=1) as xp, \
             tc.tile_pool(name="wp", bufs=1) as wpool, \
             tc.tile_pool(name="kp", bufs=1) as kp, \
             tc.tile_pool(name="bp", bufs=1) as bp, \
             tc.tile_pool(name="wu", bufs=1) as wu, \
             tc.tile_pool(name="ps", bufs=8, space="PSUM") as ps, \
             tc.tile_pool(name="ev", bufs=8) as ev, \
             tc.tile_pool(name="op", bufs=4) as op:

            kt = kp.tile([CIN, KCOLS], mmdt, tag="k")
            bt = bp.tile([128, OH], f32, tag="bias")
            xts = [xp.tile([CIN, HP * WP], mmdt, tag=f"x{n}", name=f"x{n}")
                   for n in range(BPC)]
            wts = [wpool.tile([CIN, NJ * HP * PAIRS], mmdt, tag=f"w{n}",
                              name=f"w{n}")
                   for n in range(BPC)]

            wt = None
            if WARMUP:
                wt = wu.tile([128, 128], f32, tag="warm")
                nc.vector.memset(wt[:], 0.0)

            # --- input DMAs, priority-ordered ---
            ie = [nc.sync, nc.gpsimd, nc.scalar]
            # first transform block of image 0
            for lo, hi in _splits(HEAD, 6):
                dma(ie, xts[0][:, lo:hi], x_d[0][:, lo:hi])
            # first oh half of the weights
            for lo, hi in _splits(KCOLS // 2, 10):
                dma(ie, kt[:, lo:hi], k_d[:, lo:hi])
            # rest of image 0
            for lo, hi in _splits(HP * WP - HEAD, 6):
                dma(ie, xts[0][:, HEAD + lo:HEAD + hi],
                    x_d[0][:, HEAD + lo:HEAD + hi])
            for h in range(OH):
                dma(ie, bt[:, h:h + 1], b_d[h])
            for lo, hi in _splits(KCOLS // 2, 8):
                off = KCOLS // 2
                dma(ie, kt[:, off + lo:off + hi], k_d[:, off + lo:off + hi])
            for n in range(1, BPC):
                for lo, hi in _splits(HP * WP, 6):
                    dma(ie, xts[n][:, lo:hi], x_d[n][:, lo:hi])

            # --- HAM warmup ---
            for _ in range(WARMUP):
                pw = ps.tile([128, 128], f32, tag="acc")
                nc.tensor.matmul(pw[:], wt[:], wt[:], start=True, stop=True)

            xvs = [xts[n][:].rearrange("p (r c) -> p r c", r=HP)
                   for n in range(BPC)]
            wvs = [wts[n][:].rearrange("p (j r q) -> p j r q", j=NJ, r=HP)
                   for n in range(BPC)]

            def transform(n, r0, r1):
                xv, wv = xvs[n], wvs[n]

                def dcol(k):
                    return xv[:, r0:r1, k:k + 2 * PAIRS - 1:2]

                nc.vector.tensor_sub(wv[:, 0, r0:r1, :], dcol(0), dcol(2))
                nc.vector.tensor_add(wv[:, 1, r0:r1, :], dcol(1), dcol(2))
                nc.vector.tensor_sub(wv[:, 2, r0:r1, :], dcol(2), dcol(1))
                nc.vector.tensor_sub(wv[:, 3, r0:r1, :], dcol(1), dcol(3))

            def do_strip(n, h, y0, y1):
                wv = wvs[n]
                rows = y1 - y0
                ncols = rows * PAIRS
                ms = []
                for j in range(NJ):
                    pt = ps.tile([128, ncols], f32, tag="acc",
                                 name=f"m_{n}_{h}_{y0}_{j}")
                    for kh in range(KH_EFF):
                        rhs = wv[:, j, y0 + kh:y0 + kh + rows, :]
                        off = ((h * NJ + j) * KH_EFF + kh) * 128
                        nc.tensor.matmul(pt[:], kt[:, off:off + 128], rhs,
                                         start=(kh == 0),
                                         stop=(kh == KH_EFF - 1))
                    ms.append(pt)
                # DVE may read at most one PSUM operand per op: evacuate m1
                # (bias folded in — it reaches both outputs through m1) and
                # m2 through ACT first, then combine on DVE straight into
                # the interleaved even/odd output columns.
                m1s = ev.tile([128, ncols], f32, tag="ev")
                nc.scalar.activation(m1s[:], ms[1][:],
                                     mybir.ActivationFunctionType.Identity,
                                     bias=bt[:, h:h + 1])
                m2s = ev.tile([128, ncols], f32, tag="ev")
                nc.scalar.activation(m2s[:], ms[2][:],
                                     mybir.ActivationFunctionType.Identity)
                t0 = ev.tile([128, ncols], f32, tag="ev")
                nc.vector.tensor_add(t0[:], ms[0][:], m1s[:])
                c = ev.tile([128, ncols], f32, tag="ev")
                nc.vector.tensor_sub(c[:], m1s[:], m2s[:])
                ot = op.tile([128, rows * W], f32, tag="out")
                ov = ot[:].rearrange("p (r q two) -> p r q two", r=rows, two=2)
                t0v = t0[:].rearrange("p (r q) -> p r q", r=rows)
                m2v = m2s[:].rearrange("p (r q) -> p r q", r=rows)
                cv = c[:].rearrange("p (r q) -> p r q", r=rows)
                m3v = ms[3][:].rearrange("p (r q) -> p r q", r=rows)
                nc.vector.tensor_add(ov[:, :, :, 0], t0v, m2v)
                nc.vector.tensor_sub(ov[:, :, :, 1], cv, m3v)
                last = (n == BPC - 1 and h == OH - 1 and y1 == H)
                oe = [nc.sync, nc.scalar] if last else [nc.sync, nc.gpsimd]
                for lo, hi in _splits(rows * W, 4 if last else 2):
                    dma(oe, o_d[n, h][:, y0 * W + lo:y0 * W + hi],
                        ot[:, lo:hi])

            # image 0 transforms stream in with the DMAs; image 1's are
            # emitted before its strips
            for r0, r1 in RB:
                transform(0, r0, r1)
            for h in range(OH):
                for y0, y1 in STRIPS:
                    do_strip(0, h, y0, y1)
            for r0, r1 in RB:
                transform(1, r0, r1)
            for h in range(OH):
                for y0, y1 in STRIPS:
                    do_strip(1, h, y0, y1)
    t0 = time.time()
    nc.compile()
    print(f"[kernel] bacc compile: {time.time()-t0:.1f}s", file=sys.stderr)
    return nc


def kernel(x: np.ndarray, weight: np.ndarray, bias: np.ndarray,
           P: np.ndarray) -> np.ndarray:
    global _NC_CACHE, _last_in_maps
    x = np.asarray(x, dtype=np.float32)
    weight = np.asarray(weight, dtype=np.float32)
    bias = np.asarray(bias, dtype=np.float32)
    P = np.asarray(P, dtype=np.float32)

    K = _build_kernel_np(weight, P[0])                    # (O, I, 7, 3)
    if ALGO == "wino":
        # Winograd F(2,3) over kw: 4 points per (o,i,kh);
        # device layout: [i, (oh, j, kh, o128)]
        g = K.reshape(OH, 128, CIN, KH_EFF, K_W)
        gw = np.stack([
            g[..., 0],
            (g[..., 0] + g[..., 1] + g[..., 2]) * 0.5,
            (g[..., 0] - g[..., 1] + g[..., 2]) * 0.5,
            g[..., 2],
        ], axis=1)                                # (OH, 4, 128o, CIN, KH_EFF)
        k_dev = np.ascontiguousarray(
            gw.transpose(3, 0, 1, 4, 2)
            .reshape(CIN, OH * 4 * KH_EFF * 128)).astype(_NP_DT)
    else:
        # device layout: [i, (oh, kh, kw, o128)]
        k_dev = np.ascontiguousarray(
            K.reshape(OH, 128, CIN, KH_EFF, K_W)
            .transpose(2, 0, 3, 4, 1)
            .reshape(CIN, OH * NTAPS * 128)).astype(_NP_DT)

    xpad = np.zeros((B, CIN, HP, WP), np.float32)
    xpad[:, :, PAD_H:PAD_H + H, PAD_W:PAD_W + W] = x
    xpad = xpad.reshape(B, CIN, HP * WP).astype(_NP_DT)

    b_dev = np.ascontiguousarray(bias.reshape(OH, 128, 1))

    if _NC_CACHE is None:
        t0 = time.time()
        _NC_CACHE = (_build_bass_wino() if ALGO == "wino" else _build_bass())
        print(f"[kernel] build+compile total: {time.time()-t0:.1f}s",
              file=sys.stderr)

    in_maps = [
        {"x": np.ascontiguousarray(xpad[i * BPC:(i + 1) * BPC]),
         "k": k_dev, "b": b_dev}
        for i in range(N_CORES)
    ]
    _last_in_maps = in_maps
    t0 = time.time()
    res = bass_utils.run_bass_kernel_spmd(
        _NC_CACHE, in_maps, core_ids=list(range(N_CORES)))
    print(f"[kernel] run (incl. walrus compile on first call): "
          f"{time.time()-t0:.1f}s", file=sys.stderr)
    out = np.concatenate(
        [res.results[i]["o"].reshape(BPC, COUT, H, W)
         for i in range(N_CORES)], axis=0)
    return out


# revision 23
# speedup vs baseline: 1.5720x; 1.0248x over previous
"""Dcls2_1d (dilated conv with learnable row spacings) on 8 trn2 NeuronCores.

Strategy: data-parallel over batch (16 -> 2 images/core). Host constructs the
dense (O, I, 7, 3) scattered kernel (exact port of the reference bilinear
scatter, ~0.7 MFLOP) and pads x; each core runs the conv as an implicit GEMM
contracting over C_in=128 (the partition dim), with the bias fused into the
PSUM evacuation.

Default path (DCLS_ALGO=wino, DCLS_DT=fp16): Winograd F(2,3) over the width
taps — 4 multiply-points per 2 output columns instead of 6, cutting the PE's
streamed matmul columns by 1/3. The input transform (+-1 butterflies) and the
output assembly run on the otherwise-idle DVE; PSUM is only ever read by the
scalar engine (PE-write + DVE-read on one PSUM bank is fatal on TRN2 HW).
The 7 height taps stay direct, accumulated in PSUM per 15-row strip.

Fallbacks via env: DCLS_ALGO=direct (21-tap dense GEMM), DCLS_DT=f32r
(~tf32-precision matmuls, rel err 1.4e-4 vs fp16's 4.2e-4, ~1.5x slower).

Input DMAs are priority-ordered (first strip's operands first, interleaved so
the matmul stream never catches up), issued from three engine queues in
parallel, and spread over the 16 HW DMA queues (~22 GB/s each). A short burst
of dummy matmuls warms the PE clock gate (HAM) while inputs are in flight.

Measured on trn2: ~118 us/core HW exec (PE streaming floor ~96 us), max-abs
rel err 4.2e-4 vs the fp32 reference.
"""
import os
import sys
import time

sys.path.insert(0, "/opt/trn_rl_repo")

import ml_dtypes
import numpy as np

import concourse.bass as bass
import concourse.tile as tile
from concourse import bacc, mybir
from concourse import bass_utils

# ---- problem constants (hardcoded per contract) ----
K_H, K_W = 3, 3
LIM = 2            # DIL // 2
KH_EFF = 7         # K_H + 2 * LIM
PAD_H, PAD_W = 3, 1
B, CIN, H, W = 16, 128, 64, 64
COUT = 256
N_CORES = 8
BPC = B // N_CORES                  # images per core
HP, WP = H + 2 * PAD_H, W + 2 * PAD_W   # 70, 66
NPIX = H * W                        # 4096
CHUNK = 512                         # output pixels per PSUM bank
NCHUNK = NPIX // CHUNK              # 8
RPC = CHUNK // W                    # rows per chunk: 8
NTAPS = KH_EFF * K_W                # 21
OH = COUT // 128                    # 2 halves of out channels

DT = os.environ.get("DCLS_DT", "fp16")          # f32r | fp16 | bf16 | f32
ALGO = os.environ.get("DCLS_ALGO", "wino")       # wino | direct
ORDER = os.environ.get("DCLS_ORDER", "chunk")    # chunk | tap
WARMUP = int(os.environ.get("DCLS_WARMUP", "10"))
_MM_DT = {"f32r": mybir.dt.float32r, "fp16": mybir.dt.float16,
          "bf16": mybir.dt.bfloat16, "f32": mybir.dt.float32}[DT]
_NP_DT = {"f32r": np.float32, "fp16": np.float16,
          "bf16": ml_dtypes.bfloat16, "f32": np.float32}[DT]

_NC_CACHE = None
_last_in_maps = None  # stashed for test.py's profiled re-run


def _build_kernel_np(weight: np.ndarray, P1: np.ndarray) -> np.ndarray:
    """Exact numpy port of reference.build_kernel (fp32)."""
    weight = weight.astype(np.float32, copy=False)
    kh = np.arange(K_H, dtype=np.float32)[None, None, :, None]
    pos = kh + LIM + np.clip(P1.astype(np.float32, copy=False), -LIM, LIM)
    p0 = np.floor(pos)
    frac = pos - p0
    p0i = p0.astype(np.int32)
    rng = np.arange(KH_EFF, dtype=np.int32)
    oh0 = (p0i[..., None] == rng).astype(np.float32)
    oh1 = ((p0i + 1)[..., None] == rng).astype(np.float32)
    return (
        np.einsum("oihw,oihwk->oikw", weight * (1.0 - frac), oh0)
        + np.einsum("oihw,oihwk->oikw", weight * frac, oh1)
    ).astype(np.float32)


def _splits(total, n):
    """n near-equal [lo, hi) column ranges covering [0, total)."""
    step = (total + n - 1) // n
    return [(j, min(j + step, total)) for j in range(0, total, step)]


def _build_bass():
    mmdt = _MM_DT
    f32 = mybir.dt.float32
    nc = bacc.Bacc("TRN2", target_bir_lowering=False, debug=False,
                   num_devices=N_CORES)
    x_d = nc.dram_tensor("x", [BPC, CIN, HP * WP], mmdt,
                         kind="ExternalInput").ap()
    # oh-major weight layout: [i, (oh, kh, kw, o128)]
    k_d = nc.dram_tensor("k", [CIN, OH * NTAPS * 128], mmdt,
                         kind="ExternalInput").ap()
    b_d = nc.dram_tensor("b", [OH, 128, 1], f32, kind="ExternalInput").ap()
    o_d = nc.dram_tensor("o", [BPC, OH, 128, NPIX], f32,
                         kind="ExternalOutput").ap()

    HEAD_ROWS = RPC + KH_EFF - 1            # x rows needed by first chunk: 14
    HEAD = HEAD_ROWS * WP                   # 924 cols

    # DMA descriptor issue costs ~0.6us on an engine queue; spread issues
    # over four otherwise-idle engine queues so they go out in parallel.
    _rr = [0]

    def dma(engines, dst, src):
        eng = engines[_rr[0] % len(engines)]
        _rr[0] += 1
        eng.dma_start(dst, src)

    with tile.TileContext(nc) as tc:
        with tc.tile_pool(name="xp", bufs=1) as xp, \
             tc.tile_pool(name="kp", bufs=1) as kp, \
             tc.tile_pool(name="bp", bufs=1) as bp, \
             tc.tile_pool(name="wu", bufs=1) as wu, \
             tc.tile_pool(name="ps", bufs=8, space="PSUM") as ps, \
             tc.tile_pool(name="op", bufs=4) as op:

            kt = kp.tile([CIN, OH * NTAPS * 128], mmdt, tag="k")
            bt = bp.tile([128, OH], f32, tag="bias")
            xts = [xp.tile([CIN, HP * WP], mmdt, tag=f"x{n}", name=f"x{n}")
                   for n in range(BPC)]

            # warmup tile for the PE clock (HAM) ramp: memset-fed fp32
            # (no DMA deps) so the dummy matmuls run while the real inputs
            # are still in flight; their PSUM output is never read
            wt = None
            if WARMUP:
                wt = wu.tile([128, 128], f32, tag="warm")
                nc.vector.memset(wt[:], 0.0)

            # --- input DMAs, priority-ordered, issued from 4 engines in
            # parallel, spread over the 16 HW queues ---
            ie = [nc.sync, nc.gpsimd, nc.scalar]
            # 1) first rows of image 0 (first matmul needs them + tap0 weights)
            for lo, hi in _splits(HEAD, 8):
                dma(ie, xts[0][:, lo:hi], x_d[0][:, lo:hi])
            # 2) weights for the first oh half, fine-grained so taps stream in
            for lo, hi in _splits(NTAPS * 128, 16):
                dma(ie, kt[:, lo:hi], k_d[:, lo:hi])
            # 3) rest of image 0
            for lo, hi in _splits(HP * WP - HEAD, 5):
                dma(ie, xts[0][:, HEAD + lo:HEAD + hi],
                    x_d[0][:, HEAD + lo:HEAD + hi])
            # 4) bias, second weight half, remaining images
            for h in range(OH):
                dma(ie, bt[:, h:h + 1], b_d[h])
            for lo, hi in _splits(NTAPS * 128, 8):
                off = NTAPS * 128
                dma(ie, kt[:, off + lo:off + hi], k_d[:, off + lo:off + hi])
            for n in range(1, BPC):
                for lo, hi in _splits(HP * WP, 6):
                    dma(ie, xts[n][:, lo:hi], x_d[n][:, lo:hi])

            # --- HAM warmup: dummy matmuls while inputs stream in ---
            for _ in range(WARMUP):
                pw = ps.tile([128, 128], f32, tag="acc")
                nc.tensor.matmul(pw[:], wt[:], wt[:], start=True,
                                 stop=True)

            # --- the conv ---
            def do_group(n, h, c, xv):
                pt = ps.tile([128, CHUNK], f32, tag="acc")
                y0 = c * RPC
                for t, (kh, kw) in enumerate(
                        (kh, kw) for kh in range(KH_EFF)
                        for kw in range(K_W)):
                    rhs = xv[:, y0 + kh:y0 + kh + RPC, kw:kw + W]
                    off = ((h * KH_EFF + kh) * K_W + kw) * 128
                    nc.tensor.matmul(pt[:], kt[:, off:off + 128], rhs,
                                     start=(t == 0), stop=(t == NTAPS - 1))
                ot = op.tile([128, CHUNK], f32, tag="out")
                nc.scalar.activation(ot[:], pt[:],
                                     mybir.ActivationFunctionType.Identity,
                                     bias=bt[:, h:h + 1])
                # split the store so the flush of the last chunk isn't
                # bottlenecked on a single ~22GB/s DMA queue; the very last
                # store goes 8-way on the HW queues (SW queues drain slowly)
                last = (n == BPC - 1 and h == OH - 1 and c == NCHUNK - 1)
                oe = [nc.sync, nc.scalar] if last else [nc.sync, nc.gpsimd]
                for lo, hi in _splits(CHUNK, 8 if last else 2):
                    dma(oe, o_d[n, h][:, c * CHUNK + lo:c * CHUNK + hi],
                        ot[:, lo:hi])

            def do_block_tap_outer(n, h, xv):
                pts = [ps.tile([128, CHUNK], f32, tag="acc",
                               name=f"acc_{n}_{h}_{c}")
                       for c in range(NCHUNK)]
                for t, (kh, kw) in enumerate(
                        (kh, kw) for kh in range(KH_EFF)
                        for kw in range(K_W)):
                    off = ((h * KH_EFF + kh) * K_W + kw) * 128
                    for c in range(NCHUNK):
                        rhs = xv[:, c * RPC + kh:c * RPC + kh + RPC, kw:kw + W]
                        nc.tensor.matmul(pts[c][:], kt[:, off:off + 128], rhs,
                                         start=(t == 0),
                                         stop=(t == NTAPS - 1))
                for c in range(NCHUNK):
                    ot = op.tile([128, CHUNK], f32, tag="out")
                    nc.scalar.activation(ot[:], pts[c][:],
                                         mybir.ActivationFunctionType.Identity,
                                         bias=bt[:, h:h + 1])
                    last = (n == BPC - 1 and h == OH - 1 and c == NCHUNK - 1)
                    oe = [nc.sync, nc.gpsimd]
                    for lo, hi in _splits(CHUNK, 4 if last else 2):
                        dma(oe, o_d[n, h][:, c * CHUNK + lo:c * CHUNK + hi],
                            ot[:, lo:hi])

            for n in range(BPC):
                xv = xts[n][:].rearrange("p (h w) -> p h w", h=HP)
                for h in range(OH):
                    if ORDER == "tap":
                        do_block_tap_outer(n, h, xv)
                    else:
                        for c in range(NCHUNK):
                            do_group(n, h, c, xv)
    t0 = time.time()
    nc.compile()
    print(f"[kernel] bacc compile: {time.time()-t0:.1f}s", file=sys.stderr)
    return nc


NJ = 4                       # Winograd F(2,3) points over kw
PAIRS = W // 2               # output column pairs: 32
STRIPS = [(0, 15), (15, 30), (30, 45), (45, 60), (60, 64)]
RB = [(0, 18), (18, 36), (36, 54), (54, 70)]   # input-transform row blocks


def _build_bass_wino():
    """Winograd F(2,3) over the width taps: out cols (2p, 2p+1) come from
    4 multiply-points j on input cols (2p..2p+3), so the PE streams 4/6 of
    the direct method's columns. Transforms run on the otherwise-idle
    DVE (input, output assembly) and ACT (bias) engines.

      W0 = d0-d2, W1 = d1+d2, W2 = d2-d1, W3 = d1-d3     (input, DVE)
      o_even = m0+m1+m2,  o_odd = m1-m2-m3               (output, DVE)
    """
    mmdt = _MM_DT
    f32 = mybir.dt.float32
    nc = bacc.Bacc("TRN2", target_bir_lowering=False, debug=False,
                   num_devices=N_CORES)
    x_d = nc.dram_tensor("x", [BPC, CIN, HP * WP], mmdt,
                         kind="ExternalInput").ap()
    # transformed weights: [i, (oh, j, kh, o128)]
    KCOLS = OH * NJ * KH_EFF * 128
    k_d = nc.dram_tensor("k", [CIN, KCOLS], mmdt, kind="ExternalInput").ap()
    b_d = nc.dram_tensor("b", [OH, 128, 1], f32, kind="ExternalInput").ap()
    o_d = nc.dram_tensor("o", [BPC, OH, 128, NPIX], f32,
                         kind="ExternalOutput").ap()

    _rr = [0]

    def dma(engines, dst, src):
        eng = engines[_rr[0] % len(engines)]
        _rr[0] += 1
        eng.dma_start(dst, src)

    HEAD = RB[0][1] * WP      # x cols needed by the first transform block

    with tile.TileContext(nc) as tc:
        with tc.tile_pool(name="xp", bufs=1) as xp, \
             tc.tile_pool(name="wp", bufs=1) as wpool, \
             tc.tile_pool(name="kp", bufs=1) as kp, \
             tc.tile_pool(name="bp", bufs=1) as bp, \
             tc.tile_pool(name="wu", bufs=1) as wu, \
             tc.tile_pool(name="ps", bufs=8, space="PSUM") as ps, \
             tc.tile_pool(name="ev", bufs=8) as ev, \
             tc.tile_pool(name="op", bufs=4) as op:

            kt = kp.tile([CIN, KCOLS], mmdt, tag="k")
            bt = bp.tile([128, OH], f32, tag="bias")
            xts = [xp.tile([CIN, HP * WP], mmdt, tag=f"x{n}", name=f"x{n}")
                   for n in range(BPC)]
            wts = [wpool.tile([CIN, NJ * HP * PAIRS], mmdt, tag=f"w{n}",
                              name=f"w{n}")
                   for n in range(BPC)]

            wt = None
            if WARMUP:
                wt = wu.tile([128, 128], f32, tag="warm")
                nc.vector.memset(wt[:], 0.0)

            # --- input DMAs, priority-ordered ---
            ie = [nc.sync, nc.gpsimd, nc.scalar]
            # first two transform blocks of image 0 (strip 1 consumes block
            # 1's rows ~6us after the first matmul), with the first oh half
            # of the weights (fully consumed by strip 0) interleaved so the
            # matmul stream doesn't catch up to either
            ksp = _splits(KCOLS // 2, 12)
            for lo, hi in _splits(HEAD, 6):
                dma(ie, xts[0][:, lo:hi], x_d[0][:, lo:hi])
            for lo, hi in ksp[:5]:
                dma(ie, kt[:, lo:hi], k_d[:, lo:hi])
            B1 = RB[1][1] * WP
            for lo, hi in _splits(B1 - HEAD, 4):
                dma(ie, xts[0][:, HEAD + lo:HEAD + hi],
                    x_d[0][:, HEAD + lo:HEAD + hi])
            for lo, hi in ksp[5:]:
                dma(ie, kt[:, lo:hi], k_d[:, lo:hi])
            # rest of image 0
            for lo, hi in _splits(HP * WP - B1, 5):
                dma(ie, xts[0][:, B1 + lo:B1 + hi],
                    x_d[0][:, B1 + lo:B1 + hi])
            for h in range(OH):
                dma(ie, bt[:, h:h + 1], b_d[h])
            for lo, hi in _splits(KCOLS // 2, 8):
                off = KCOLS // 2
                dma(ie, kt[:, off + lo:off + hi], k_d[:, off + lo:off + hi])
            for n in range(1, BPC):
                for lo, hi in _splits(HP * WP, 6):
                    dma(ie, xts[n][:, lo:hi], x_d[n][:, lo:hi])

            # --- HAM warmup ---
            for _ in range(WARMUP):
                pw = ps.tile([128, 128], f32, tag="acc")
                nc.tensor.matmul(pw[:], wt[:], wt[:], start=True, stop=True)

            xvs = [xts[n][:].rearrange("p (r c) -> p r c", r=HP)
                   for n in range(BPC)]
            wvs = [wts[n][:].rearrange("p (j r q) -> p j r q", j=NJ, r=HP)
                   for n in range(BPC)]

            def transform(n, r0, r1):
                xv, wv = xvs[n], wvs[n]

                def dcol(k):
                    return xv[:, r0:r1, k:k + 2 * PAIRS - 1:2]

                nc.vector.tensor_sub(wv[:, 0, r0:r1, :], dcol(0), dcol(2))
                nc.vector.tensor_add(wv[:, 1, r0:r1, :], dcol(1), dcol(2))
                nc.vector.tensor_sub(wv[:, 2, r0:r1, :], dcol(2), dcol(1))
                nc.vector.tensor_sub(wv[:, 3, r0:r1, :], dcol(1), dcol(3))

            def do_strip(n, h, y0, y1):
                wv = wvs[n]
                rows = y1 - y0
                ncols = rows * PAIRS
                ms = []
                for j in range(NJ):
                    pt = ps.tile([128, ncols], f32, tag="acc",
                                 name=f"m_{n}_{h}_{y0}_{j}")
                    for kh in range(KH_EFF):
                        rhs = wv[:, j, y0 + kh:y0 + kh + rows, :]
                        off = ((h * NJ + j) * KH_EFF + kh) * 128
                        nc.tensor.matmul(pt[:], kt[:, off:off + 128], rhs,
                                         start=(kh == 0),
                                         stop=(kh == KH_EFF - 1))
                    ms.append(pt)
                # DVE may read at most one PSUM operand per op: evacuate m1
                # (bias folded in — it reaches both outputs through m1) and
                # m2 through ACT first, then combine on DVE straight into
                # the interleaved even/odd output columns.
                m1s = ev.tile([128, ncols], f32, tag="ev")
                nc.scalar.activation(m1s[:], ms[1][:],
                                     mybir.ActivationFunctionType.Identity,
                                     bias=bt[:, h:h + 1])
                m2s = ev.tile([128, ncols], f32, tag="ev")
                nc.scalar.activation(m2s[:], ms[2][:],
                                     mybir.ActivationFunctionType.Identity)
                t0 = ev.tile([128, ncols], f32, tag="ev")
                nc.vector.tensor_add(t0[:], ms[0][:], m1s[:])
                c = ev.tile([128, ncols], f32, tag="ev")
                nc.vector.tensor_sub(c[:], m1s[:], m2s[:])
                ot = op.tile([128, rows * W], f32, tag="out")
                ov = ot[:].rearrange("p (r q two) -> p r q two", r=rows, two=2)
                t0v = t0[:].rearrange("p (r q) -> p r q", r=rows)
                m2v = m2s[:].rearrange("p (r q) -> p r q", r=rows)
                cv = c[:].rearrange("p (r q) -> p r q", r=rows)
                m3v = ms[3][:].rearrange("p (r q) -> p r q", r=rows)
                nc.vector.tensor_add(ov[:, :, :, 0], t0v, m2v)
                nc.vector.tensor_sub(ov[:, :, :, 1], cv, m3v)
                last = (n == BPC - 1 and h == OH - 1 and y1 == H)
                oe = [nc.sync, nc.scalar] if last else [nc.sync, nc.gpsimd]
                for lo, hi in _splits(rows * W, 4 if last else 2):
                    dma(oe, o_d[n, h][:, y0 * W + lo:y0 * W + hi],
                        ot[:, lo:hi])

            # image 0 transforms stream in with the DMAs; image 1's are
            # emitted before its strips
            for r0, r1 in RB:
                transform(0, r0, r1)
            for h in range(OH):
                for y0, y1 in STRIPS:
                    do_strip(0, h, y0, y1)
            for r0, r1 in RB:
                transform(1, r0, r1)
            for h in range(OH):
                for y0, y1 in STRIPS:
                    do_strip(1, h, y0, y1)
    t0 = time.time()
    nc.compile()
    print(f"[kernel] bacc compile: {time.time()-t0:.1f}s", file=sys.stderr)
    return nc


def kernel(x: np.ndarray, weight: np.ndarray, bias: np.ndarray,
           P: np.ndarray) -> np.ndarray:
    global _NC_CACHE, _last_in_maps
    x = np.asarray(x, dtype=np.float32)
    weight = np.asarray(weight, dtype=np.float32)
    bias = np.asarray(bias, dtype=np.float32)
    P = np.asarray(P, dtype=np.float32)

    K = _build_kernel_np(weight, P[0])                    # (O, I, 7, 3)
    if ALGO == "wino":
        # Winograd F(2,3) over kw: 4 points per (o,i,kh);
        # device layout: [i, (oh, j, kh, o128)]
        g = K.reshape(OH, 128, CIN, KH_EFF, K_W)
        gw = np.stack([
            g[..., 0],
            (g[..., 0] + g[..., 1] + g[..., 2]) * 0.5,
            (g[..., 0] - g[..., 1] + g[..., 2]) * 0.5,
            g[..., 2],
        ], axis=1)                                # (OH, 4, 128o, CIN, KH_EFF)
        k_dev = np.ascontiguousarray(
            gw.transpose(3, 0, 1, 4, 2)
            .reshape(CIN, OH * 4 * KH_EFF * 128)).astype(_NP_DT)
    else:
        # device layout: [i, (oh, kh, kw, o128)]
        k_dev = np.ascontiguousarray(
            K.reshape(OH, 128, CIN, KH_EFF, K_W)
            .transpose(2, 0, 3, 4, 1)
            .reshape(CIN, OH * NTAPS * 128)).astype(_NP_DT)

    xpad = np.zeros((B, CIN, HP, WP), np.float32)
    xpad[:, :, PAD_H:PAD_H + H, PAD_W:PAD_W + W] = x
    xpad = xpad.reshape(B, CIN, HP * WP).astype(_NP_DT)

    b_dev = np.ascontiguousarray(bias.reshape(OH, 128, 1))

    if _NC_CACHE is None:
        t0 = time.time()
        _NC_CACHE = (_build_bass_wino() if ALGO == "wino" else _build_bass())
        print(f"[kernel] build+compile total: {time.time()-t0:.1f}s",
              file=sys.stderr)

    in_maps = [
        {"x": np.ascontiguousarray(xpad[i * BPC:(i + 1) * BPC]),
         "k": k_dev, "b": b_dev}
        for i in range(N_CORES)
    ]
    _last_in_maps = in_maps
    t0 = time.time()
    last_exc = None
    for attempt in range(3):
        try:
            res = bass_utils.run_bass_kernel_spmd(
                _NC_CACHE, in_maps, core_ids=list(range(N_CORES)))
            break
        except Exception as e:  # transient device hiccup: retry
            last_exc = e
            print(f"[kernel] run attempt {attempt} failed: {e!r}; retrying",
                  file=sys.stderr)
            time.sleep(5)
    else:
        raise last_exc
    print(f"[kernel] run (incl. walrus compile on first call): "
          f"{time.time()-t0:.1f}s", file=sys.stderr)
    out = np.concatenate(
        [res.results[i]["o"].reshape(BPC, COUT, H, W)
         for i in range(N_CORES)], axis=0)
    return out
